# revision 14
# baseline (speedup 1.0000x reference)
"""Trainium2 Bass kernel for nn_LowRankProjection: y = (spikes @ V) @ U.T.

Strategy (data-parallel over batch, 8 cores; fp16 I/O, fp32 PSUM accum —
harness gate is rel_err < 2e-2, fp16 pipeline measures ~5e-4):
  - Host pre-layouts:
      sT2  = spikes.T shard packed as [2, NCH, 128, KPER*BSH/2] — the exact
             per-DMA tile layout, so every input DMA reads fully-contiguous
             8 KiB runs per partition (full DMA rate, no strided descriptors)
      Vd   = V rearranged to [128, (N_PRE/128)*R] so lhsT chunks are slices
      Ut   = U.T [R, N_POST]
  - Device, per core, two batch halves software-pipelined so half B's
    input stream overlaps half A's phase 2 (hides the z-dependency stall):
      phase 1: z[r, b] += V_k.T @ sT_k over 128 k-chunks into one PSUM bank
      phase 2: y[b_chunk, n] = z_chunk.T @ Ut  (fp16 out)
    PSUM->SBUF downcast copies alternate DVE/Activation; input loads on
    sync's HWDGE queue, output stores on gpsimd's SWDGE queue so neither
    stream's issue order can block the other.
  - Memory-bound: per core 16 MiB in + 16 MiB out + ~1.3 MiB weights.
"""

import numpy as np

import concourse.bacc as bacc
import concourse.mybir as mybir
import concourse.tile as tile
from concourse.bass_utils import run_bass_kernel_spmd

B, N_PRE, N_POST, R = 4096, 16384, 16384, 32
N_CORES = 8
BSH = B // N_CORES  # 512 batch rows per core
NH = 2  # batch halves per core (software pipeline depth)
BH = BSH // NH  # 256 batch rows per half
P = 128
KC = N_PRE // P  # 128 contraction chunks
F32 = mybir.dt.float32
F16 = mybir.dt.float16

KPER = 16  # k-chunks per input DMA (1 MiB fp16 per half)
NPC = 8  # 512-wide output chunks per output DMA (1 MiB fp16)
# Deferred half-A o_tiles: {phase1-chunk-index-of-half-B: tiles emitted just
# before that chunk, -1: after the last chunk}.
DEFER_PLAN = {5: 1, 6: 1, 7: 1, -1: 1}


def _body(tc, y, sT2, vd, ut):
    nc = tc.nc
    with (
        tc.tile_pool(name="w", bufs=1) as wpool,
        tc.tile_pool(name="s", bufs=3) as spool,
        tc.tile_pool(name="o", bufs=3) as opool,
        tc.tile_pool(name="z", bufs=2) as zpool,
        tc.tile_pool(name="zps", bufs=2, space="PSUM") as zpspool,
        tc.tile_pool(name="yps", bufs=4, space="PSUM") as ypspool,
    ):
        # Weights ride the gpsimd (SWDGE) queue so they don't serialize
        # ahead of the spikes stream in sync's HWDGE FIFO.
        v_sb = wpool.tile([P, KC * R], F16)
        nc.gpsimd.dma_start(v_sb[:], vd[:])
        ut_sb = wpool.tile([R, N_POST], F16)
        nc.gpsimd.dma_start(ut_sb[:], ut[:])

        def emit_phase1_chunk(h, zps, ci):
            s_tile = spool.tile([P, KPER, BH], F16)
            nc.sync.dma_start(
                s_tile[:], sT2[h, ci].rearrange("p (a b) -> p a b", a=KPER)
            )
            for j in range(KPER):
                k = ci * KPER + j
                nc.tensor.matmul(
                    zps[:],
                    v_sb[:, k * R : (k + 1) * R],
                    s_tile[:, j, :],
                    start=(k == 0),
                    stop=(k == KC - 1),
                )

        def emit_otile(h, zt, ot):
            bi, grp = divmod(ot, N_POST // (512 * NPC))
            o_tile = opool.tile([P, NPC * 512], F16)
            for j in range(NPC):
                n0 = grp * NPC * 512 + j * 512
                yp = ypspool.tile([P, 512], F32)
                nc.tensor.matmul(
                    yp[:],
                    zt[:, bi * P : (bi + 1) * P],
                    ut_sb[:, n0 : n0 + 512],
                    start=True,
                    stop=True,
                )
                if j % 2 == 0:
                    nc.vector.tensor_copy(o_tile[:, j * 512 : (j + 1) * 512], yp[:])
                else:
                    nc.scalar.copy(o_tile[:, j * 512 : (j + 1) * 512], yp[:])
            nc.gpsimd.dma_start(
                y[
                    h * BH + bi * P : h * BH + (bi + 1) * P,
                    grp * NPC * 512 : (grp + 1) * NPC * 512,
                ],
                o_tile[:],
            )

        # Two batch halves, software pipelined: half B's input stream hides
        # half A's phase 2. The last few o_tiles of half A are deferred and
        # emitted between half B's late phase-1 chunks (per DEFER_PLAN:
        # {chunk_index: n_tiles_before_that_chunk}, -1 = after last chunk)
        # so their output transfers cover the z(B)-dependency stall after
        # the final input DMA.
        NOT = (BH // P) * (N_POST // (512 * NPC))  # o_tiles per half
        NCH = KC // KPER  # phase-1 chunks per half

        zps0 = zpspool.tile([R, BH], F32, tag="z0")
        for ci in range(NCH):
            emit_phase1_chunk(0, zps0, ci)
        zt0 = zpool.tile([R, BH], F16, tag="zt0")
        nc.vector.tensor_copy(zt0[:], zps0[:])
        n_defer = sum(DEFER_PLAN.values())
        for ot in range(NOT - n_defer):
            emit_otile(0, zt0, ot)

        deferred = iter(range(NOT - n_defer, NOT))
        zps1 = zpspool.tile([R, BH], F32, tag="z1")
        for ci in range(NCH):
            for _ in range(DEFER_PLAN.get(ci, 0)):
                emit_otile(0, zt0, next(deferred))
            emit_phase1_chunk(1, zps1, ci)
        for _ in range(DEFER_PLAN.get(-1, 0)):
            emit_otile(0, zt0, next(deferred))
        zt1 = zpool.tile([R, BH], F16, tag="zt1")
        nc.vector.tensor_copy(zt1[:], zps1[:])
        for ot in range(NOT):
            emit_otile(1, zt1, ot)


_NC_CACHE = None


def _build():
    global _NC_CACHE
    if _NC_CACHE is None:
        nc = bacc.Bacc(
            "TRN2", target_bir_lowering=False, debug=False, num_devices=N_CORES
        )
        sT2 = nc.dram_tensor(
            "sT2", [NH, KC // KPER, P, KPER * BH], F16, kind="ExternalInput"
        ).ap()
        vd = nc.dram_tensor("Vd", [P, KC * R], F16, kind="ExternalInput").ap()
        ut = nc.dram_tensor("Ut", [R, N_POST], F16, kind="ExternalInput").ap()
        y = nc.dram_tensor("y", [BSH, N_POST], F16, kind="ExternalOutput").ap()
        with tile.TileContext(nc) as tc:
            _body(tc, y, sT2, vd, ut)
        nc.compile()
        _NC_CACHE = nc
    return _NC_CACHE


def _prep_inputs(spikes, U, V):
    spikes = np.asarray(spikes, dtype=np.float32)
    sT = np.ascontiguousarray(spikes.T).astype(np.float16)  # [N_PRE, B]
    vd = np.ascontiguousarray(
        np.asarray(V, dtype=np.float32)
        .reshape(KC, P, R)
        .transpose(1, 0, 2)
        .reshape(P, KC * R)
    ).astype(np.float16)
    ut = np.ascontiguousarray(np.asarray(U, dtype=np.float32).T).astype(
        np.float16
    )  # [R, N_POST]
    in_maps = []
    for c in range(N_CORES):
        sh = sT[:, c * BSH : (c + 1) * BSH]  # [N_PRE, BSH]
        # Pack into the exact per-DMA tile layout [NH, NCH, P, KPER*BH] so
        # every input DMA reads fully-contiguous 8 KiB runs per partition:
        # sh[(ci*KPER+j)*P + p, h*BH + b] -> sT2[h, ci, p, j*BH + b].
        sh2 = np.ascontiguousarray(
            sh.reshape(KC // KPER, KPER, P, NH, BH)
            .transpose(3, 0, 2, 1, 4)
            .reshape(NH, KC // KPER, P, KPER * BH)
        )
        in_maps.append({"sT2": sh2, "Vd": vd, "Ut": ut})
    return in_maps


def _run(spikes, U, V, **run_kwargs):
    nc = _build()
    in_maps = _prep_inputs(spikes, U, V)
    res = run_bass_kernel_spmd(nc, in_maps, list(range(N_CORES)), **run_kwargs)
    y = np.concatenate(
        [res.results[c]["y"].astype(np.float32) for c in range(N_CORES)], axis=0
    )
    return y, res


def kernel(spikes, U, V, mask_row_ptr=None, mask_col_idx=None, mask_values=None):
    y, _ = _run(spikes, U, V)
    return y
